# revision 1
# baseline (speedup 1.0000x reference)
"""GQA attention decode step (B=8, S=16, D=4096, H=32, KVH=8, HD=128) on 8
Trainium2 NeuronCores, tensor-parallel over heads.

Core c owns q-heads [4c, 4c+4), kv-head c, wo rows [512c, 512(c+1)).
Each core computes a full (128, 4096) partial output; the host sums the 8
partials (the all-reduce) and reshapes to (B, S, D).
"""

import os
import sys

import numpy as np

sys.path.insert(0, "/opt/trn_rl_repo")

import concourse.bacc as bacc  # noqa: E402
import concourse.mybir as mybir  # noqa: E402
import concourse.tile as tile  # noqa: E402
from concourse.tile import add_dep_helper  # noqa: E402
from concourse import bass_utils  # noqa: E402

D = 4096
H = 32
KVH = 8
HD = 128
NCORES = 8
HQ = H // NCORES          # 4 q heads per core
CW = HQ * HD              # 512 = per-core q width / wo row count
KVW = 2 * HD              # 256 = fused [k|v] projection width

TRACE = False
USE_BF16C = True          # hi/lo bf16 compensated qkv+wo matmuls (same DMA bytes, 3 vs 4 cyc/row)
_LAST = None              # last BassKernelResults (for test.py)
_BUILD_CACHE = {}

F32 = mybir.dt.float32


def _build(L, B, S, reps=1):
    """Build the per-core Bass program (identical across cores; SPMD).

    reps>1 replicates the whole body (for on-device benchmarking: one
    dispatch runs the kernel reps times back-to-back)."""
    T = B * S
    assert T == 128, "kernel assumes 128 tokens (B*S)"
    nfull, rem = divmod(L, 128)
    CS = [128] * nfull + ([rem] if rem else [])  # cache kv-chunk sizes
    ncache = len(CS)
    tpb = ncache                                  # cache tiles per batch
    NCHUNK = ncache + 1                           # + new-kv chunk (S wide)
    QW = HQ * S                                   # 64 score columns (4 heads x 16 tok)
    SCW = NCHUNK * QW                             # scoresT psum width per batch
    NKT = D // 128                                # 32 contraction tiles
    WCH = 8                                       # k-tiles per weight DMA chunk
    NWCH = NKT // WCH                             # 4 weight chunks
    scale = 1.0 / float(np.sqrt(HD))

    nc = bacc.Bacc("TRN2", target_bir_lowering=False, debug=False)
    BF = mybir.dt.bfloat16
    WMUL = 2 if USE_BF16C else 1
    WT = BF if USE_BF16C else F32

    xt_d = nc.dram_tensor("xt", (128, WMUL * D), WT, kind="ExternalInput")
    wq_d = nc.dram_tensor("wq", (128, WMUL * NKT * CW), WT, kind="ExternalInput")
    wkv_d = nc.dram_tensor("wkv", (128, WMUL * NKT * KVW), WT, kind="ExternalInput")
    wo_d = nc.dram_tensor("wo", (128, WMUL * HQ * D), WT, kind="ExternalInput")
    crep_d = nc.dram_tensor("crep", (128, (HQ + 1) * HD), F32, kind="ExternalInput")
    shalf_d = nc.dram_tensor("shalf", (128, (HQ + 1) * HD // 2), F32, kind="ExternalInput")
    ident_d = nc.dram_tensor("ident", (128, 128), F32, kind="ExternalInput")
    if ncache:
        keyst_d = nc.dram_tensor("keyst", (128, WMUL * B * tpb * 128), WT, kind="ExternalInput")
        vals_d = nc.dram_tensor("vals", (128, WMUL * B * tpb * (HD + 1)), WT, kind="ExternalInput")
    out_d = nc.dram_tensor("out", (T, D), F32, kind="ExternalOutput")

    with tile.TileContext(nc) as tc:
      for _rep in range(reps):
        with tc.tile_pool(name=f"const{_rep}", bufs=1) as cpool:
            crep = cpool.tile([128, (HQ + 1) * HD], F32)
            nc.sync.dma_start(crep[:], crep_d[:])
            shalf = cpool.tile([128, (HQ + 1) * HD // 2], F32)
            nc.sync.dma_start(shalf[:], shalf_d[:])
            ident = cpool.tile([128, 128], F32)
            nc.sync.dma_start(ident[:], ident_d[:])
            # long-lived intermediates
            qt = cpool.tile([128, CW], F32)            # (hd, h*128 + tok)
            kt = cpool.tile([128, 128], F32)           # (hd, tok)
            vreb = cpool.tile([S, B * (HD + 1)], F32)  # new-v, partition-rebased
            vrebH = cpool.tile([S, B * (HD + 1)], BF)
            vrebL = cpool.tile([S, B * (HD + 1)], BF)
            qtH = cpool.tile([128, CW], BF)
            qtL = cpool.tile([128, CW], BF)
            ktH = cpool.tile([128, 128], BF)
            ktL = cpool.tile([128, 128], BF)
            ctxt = cpool.tile([128, B * QW], F32)      # (hd, b*64 + h*16 + s)

            # ---------------- phase 1: qkv projections ----------------
            with (
                tc.tile_pool(name=f"xtp{_rep}", bufs=3) as xtp,
                tc.tile_pool(name=f"wqp{_rep}", bufs=3) as wqp,
                tc.tile_pool(name=f"wkvp{_rep}", bufs=3) as wkvp,
                tc.tile_pool(name="pqkv", bufs=1, space="PSUM") as pqkv,
            ):
                q_ps = pqkv.tile([128, CW], F32, tag="q")
                kv_ps = pqkv.tile([128, KVW], F32, tag="kv")
                # hi/lo combos: (xtH,wH), (xtH,wL), (xtL,wH); lo halves sit in
                # the second half of each chunk's columns
                combos = [(0, 0), (0, 1), (1, 0)] if USE_BF16C else [(0, 0)]
                XW, QWW, KWW = WCH * 128, WCH * CW, WCH * KVW
                for ch in range(NWCH):
                    xt_t = xtp.tile([128, WMUL * XW], WT, tag="xt")
                    nc.sync.dma_start(xt_t[:], xt_d[:, ch * WMUL * XW:(ch + 1) * WMUL * XW])
                    wq_t = wqp.tile([128, WMUL * QWW], WT, tag="wq")
                    nc.sync.dma_start(wq_t[:], wq_d[:, ch * WMUL * QWW:(ch + 1) * WMUL * QWW])
                    wkv_t = wkvp.tile([128, WMUL * KWW], WT, tag="wkv")
                    nc.sync.dma_start(wkv_t[:], wkv_d[:, ch * WMUL * KWW:(ch + 1) * WMUL * KWW])
                    for j in range(WCH):
                        k = ch * WCH + j
                        for ci, (a, w) in enumerate(combos):
                            first = (k == 0 and ci == 0)
                            last = (k == NKT - 1 and ci == len(combos) - 1)
                            xs = a * XW + j * 128
                            nc.tensor.matmul(
                                q_ps[:], xt_t[:, xs:xs + 128],
                                wq_t[:, w * QWW + j * CW:w * QWW + (j + 1) * CW],
                                start=first, stop=last)
                            nc.tensor.matmul(
                                kv_ps[:], xt_t[:, xs:xs + 128],
                                wkv_t[:, w * KWW + j * KVW:w * KWW + (j + 1) * KVW],
                                start=first, stop=last)

                # ---------------- phase 2: rotary (DVE) ----------------
                # rot = t*crep ; rot_even -= t_odd*shalf ; rot_odd += t_even*shalf
                with tc.tile_pool(name=f"rotp{_rep}", bufs=1) as rotp:
                    W = (HQ + 1) * HD  # 640 = 4 q heads + 1 k head
                    rot = rotp.tile([128, W], F32, tag="rot")
                    tmpa = rotp.tile([128, W // 2], F32, tag="tmpa")
                    tmpb = rotp.tile([128, W // 2], F32, tag="tmpb")
                    nc.vector.tensor_mul(rot[:, 0:CW], q_ps[:], crep[:, 0:CW])
                    nc.vector.tensor_mul(rot[:, CW:W], kv_ps[:, 0:HD], crep[:, CW:W])
                    q_v = q_ps[:].rearrange("p (a two) -> p a two", two=2)
                    k_v = kv_ps[:, 0:HD].rearrange("p (a two) -> p a two", two=2)
                    nc.vector.tensor_mul(tmpa[:, 0:CW // 2], q_v[:, :, 1], shalf[:, 0:CW // 2])
                    nc.vector.tensor_mul(tmpa[:, CW // 2:], k_v[:, :, 1], shalf[:, CW // 2:])
                    nc.vector.tensor_mul(tmpb[:, 0:CW // 2], q_v[:, :, 0], shalf[:, 0:CW // 2])
                    nc.vector.tensor_mul(tmpb[:, CW // 2:], k_v[:, :, 0], shalf[:, CW // 2:])
                    rot_v = rot[:].rearrange("p (a two) -> p a two", two=2)
                    nc.vector.tensor_sub(rot_v[:, :, 0], rot_v[:, :, 0], tmpa[:])
                    nc.vector.tensor_add(rot_v[:, :, 1], rot_v[:, :, 1], tmpb[:])

                    # -------- phase 3: transposes + new-v staging --------
                    vaug = rotp.tile([128, HD + 1], F32, tag="vaug")
                    # qt is stored batch-major: column b*64 + h*16 + s, so the
                    # scores rhs is a contiguous (128, 64) slice per batch
                    qt_v = qt[:].rearrange("p (b h s) -> p b h s", b=B, h=HQ)
                    vt = rotp.tile([128, 128], F32, tag="vt")
                    with tc.tile_pool(name=f"ptr{_rep}", bufs=2, space="PSUM") as ptr:
                        for h in range(HQ):
                            tp = ptr.tile([128, 128], F32, tag="tr", name=f"tr{h}")
                            nc.tensor.transpose(tp[:], rot[:, h * 128:(h + 1) * 128], ident[:])
                            nc.vector.tensor_copy(
                                qt_v[:, :, h, :],
                                tp[:].rearrange("p (b s) -> p b s", b=B))
                        tp = ptr.tile([128, 128], F32, tag="tr", name="trk")
                        nc.tensor.transpose(tp[:], rot[:, CW:W], ident[:])
                        nc.vector.tensor_copy(kt[:], tp[:])
                        # new-v rebased to partition 0 via double transpose (no
                        # DMA: the sbuf->sbuf route queues behind ~20MB of
                        # loads and stalls PV(0) -> in-order PE for ~14us)
                        nc.vector.tensor_copy(vaug[:, 0:HD], kv_ps[:, HD:KVW])
                        nc.vector.memset(vaug[:, HD:HD + 1], 1.0)
                        tpv = ptr.tile([128, 128], F32, tag="tr", name="trv")
                        nc.tensor.transpose(tpv[:], vaug[:, 0:HD], ident[:])
                        nc.vector.tensor_copy(vt[:], tpv[:])  # (hd, tok)
                        for b in range(B):
                            tq = ptr.tile([S, 128], F32, tag="trb", name=f"trb{b}")
                            nc.tensor.transpose(tq[:], vt[:, b * S:(b + 1) * S], ident[:])
                            nc.vector.tensor_copy(
                                vreb[:, b * (HD + 1):b * (HD + 1) + HD], tq[:])
                            nc.vector.memset(
                                vreb[:, b * (HD + 1) + HD:(b + 1) * (HD + 1)], 1.0)
                        nc.vector.tensor_copy(vrebH[:], vreb[:])
                        nc.vector.tensor_sub(vrebL[:], vreb[:], vrebH[:])
                        nc.vector.tensor_copy(qtH[:], qt[:])
                        nc.vector.tensor_sub(qtL[:], qt[:], qtH[:])
                        nc.vector.tensor_copy(ktH[:], kt[:])
                        nc.vector.tensor_sub(ktL[:], kt[:], ktH[:])

            # ---------------- phase 4: attention per batch ----------------
            # wop opens alongside the attention pools so its SBUF space is
            # disjoint from kc/vc (else wo DMAs inherit WAR deps on them)
            with tc.tile_pool(name=f"wop{_rep}", bufs=8) as wop:
              with (
                  tc.tile_pool(name=f"kcp{_rep}", bufs=8) as kcp,
                  tc.tile_pool(name=f"vcp{_rep}", bufs=8) as vcp,
                  tc.tile_pool(name=f"expp{_rep}", bufs=2) as expp,
                  tc.tile_pool(name=f"psc{_rep}", bufs=2, space="PSUM") as psc,
                  tc.tile_pool(name=f"pctx{_rep}", bufs=2, space="PSUM") as pctx,
                  tc.tile_pool(name=f"pctr{_rep}", bufs=2, space="PSUM") as pctr,
              ):
                  wo_tiles = {}
                  last_kv = {"inst": None}
                  WOW = HQ * 512
                  NCH_WO = D // 512

                  def issue_wo(n):
                      wo_t = wop.tile([128, WMUL * WOW], WT, tag="wo", name=f"wo{n}")
                      di = nc.sync.dma_start(
                          wo_t[:], wo_d[:, n * WMUL * WOW:(n + 1) * WMUL * WOW])
                      if last_kv["inst"] is not None:
                          # strict back-pressure: wo bandwidth must not delay
                          # the kv stream (lanes bandwidth-share otherwise)
                          add_dep_helper(di.ins, last_kv["inst"].ins,
                                         reason="wo after last kv")
                      wo_tiles[n] = wo_t

                  st = {}

                  def do_pv(i):
                      # PV + normalize for batch i (exp already done), bf16
                      # hi/lo compensated: eH@vH + eH@vL + eL@vH
                      exHi, exLi, vci = st[i]["exH"], st[i]["exL"], st[i]["vc"]
                      VB = WMUL * (HD + 1)
                      ctx = pctx.tile([QW, HD + 1], F32, tag="ctx", name=f"ctx{i}")
                      pv_combos = [(exHi, 0), (exHi, 1), (exLi, 0)] if USE_BF16C \
                          else [(exHi, 0)]
                      first = True
                      for j, cs in enumerate(CS):
                          for el, w in pv_combos:
                              nc.tensor.matmul(
                                  ctx[:], el[0:cs, j * QW:(j + 1) * QW],
                                  vci[0:cs, j * VB + w * (HD + 1):
                                      j * VB + (w + 1) * (HD + 1)],
                                  start=first, stop=False)
                              first = False
                      vreb_combos = [(exHi, vrebH), (exHi, vrebL), (exLi, vrebH)] \
                          if USE_BF16C else [(exHi, vreb)]
                      for ci, (el, vr) in enumerate(vreb_combos):
                          nc.tensor.matmul(
                              ctx[:], el[0:S, ncache * QW:NCHUNK * QW],
                              vr[:, i * (HD + 1):(i + 1) * (HD + 1)],
                              start=(ncache == 0 and ci == 0),
                              stop=(ci == len(vreb_combos) - 1))
                      rc = expp.tile([QW, 1], F32, tag="rc", name=f"rc{i}")
                      nc.vector.reciprocal(rc[:], ctx[:, HD:HD + 1])
                      cn = expp.tile([QW, HD], F32, tag="cn", name=f"cn{i}")
                      nc.vector.tensor_scalar_mul(cn[:], ctx[:, 0:HD], rc[:])
                      st[i]["cn"] = cn

                  def do_tr(i):
                      # transpose + scatter into ctxt for batch i
                      ct = pctr.tile([128, QW], F32, tag="ct", name=f"ct{i}")
                      nc.tensor.transpose(ct[:], st[i]["cn"][:], ident[0:QW, 0:QW])
                      nc.vector.tensor_copy(
                          ctxt[:].rearrange("p (h b s) -> p h b s", h=HQ, b=B)[:, :, i, :],
                          ct[:].rearrange("p (h s) -> p h s", h=HQ))

                  for b in range(B):
                      if ncache:
                          KB = WMUL * 128
                          kc_t = kcp.tile([128, tpb * KB], WT, tag="kc")
                          nc.sync.dma_start(
                              kc_t[:], keyst_d[:, b * tpb * KB:(b + 1) * tpb * KB])
                          VB = WMUL * (HD + 1)
                          vc_t = vcp.tile([128, tpb * VB], WT, tag="vc")
                          vdi = nc.sync.dma_start(
                              vc_t[:], vals_d[:, b * tpb * VB:(b + 1) * tpb * VB])
                          if b == B - 1:
                              last_kv["inst"] = vdi
                      else:
                          kc_t = vc_t = None
                      if b == B - 1:
                          # wo stream starts after ALL kv: just-in-time arrival
                          for n in range(4):
                              issue_wo(n)
                      qbH = qtH[:, b * QW:(b + 1) * QW]
                      qbL = qtL[:, b * QW:(b + 1) * QW]

                      sc = psc.tile([128, SCW], F32, tag="sc", name=f"sc{b}")
                      # pre-fill columns of partial chunks so garbage partitions
                      # exp() to 0; the matmuls below overwrite the valid rows
                      nc.vector.memset(sc[:, ncache * QW:NCHUNK * QW], -1e30)
                      for j, cs in enumerate(CS):
                          if cs < 128:
                              nc.vector.memset(sc[:, j * QW:(j + 1) * QW], -1e30)
                      KB = WMUL * 128
                      for j, cs in enumerate(CS):
                          combos = [(0, qbH), (0, qbL), (1, qbH)] if USE_BF16C \
                              else [(0, qbH)]
                          for ci, (w, qb) in enumerate(combos):
                              nc.tensor.matmul(
                                  sc[0:cs, j * QW:(j + 1) * QW],
                                  kc_t[:, j * KB + w * 128:j * KB + w * 128 + cs],
                                  qb,
                                  start=(ci == 0), stop=(ci == len(combos) - 1))
                      # new-kv chunk
                      ncomb = [(ktH, qbH), (ktH, qbL), (ktL, qbH)] if USE_BF16C \
                          else [(ktH, qbH)]
                      for ci, (ktx, qb) in enumerate(ncomb):
                          nc.tensor.matmul(
                              sc[0:S, ncache * QW:NCHUNK * QW],
                              ktx[:, b * S:(b + 1) * S], qb,
                              start=(ci == 0), stop=(ci == len(ncomb) - 1))

                      # software pipeline: PE stays busy with prior batches'
                      # PV/transpose while ACT runs this batch's exp
                      if b >= 1:
                          do_pv(b - 1)
                      if b >= 2:
                          do_tr(b - 2)

                      ex = expp.tile([128, SCW], F32, tag="ex", name=f"ex{b}")
                      nc.scalar.activation(ex[:], sc[:], mybir.ActivationFunctionType.Exp,
                                           scale=scale)
                      exH = expp.tile([128, SCW], BF, tag="exH", name=f"exH{b}")
                      nc.vector.tensor_copy(exH[:], ex[:])
                      exL = expp.tile([128, SCW], BF, tag="exL", name=f"exL{b}")
                      nc.vector.tensor_sub(exL[:], ex[:], exH[:])
                      st[b] = dict(exH=exH, exL=exL, vc=vc_t)

                  do_pv(B - 1)
                  do_tr(B - 2)
                  do_tr(B - 1)

              # ---------------- phase 5: wo projection ----------------
              # wo is chunked by OUTPUT columns: each 1MB chunk holds all 4
              # h-blocks for one 512-col output slice, so its psum completes
              # and streams out immediately
              with (
                  tc.tile_pool(name=f"outp{_rep}", bufs=4) as outp,
                  tc.tile_pool(name=f"pwo{_rep}", bufs=3, space="PSUM") as pwo,
              ):
                  if USE_BF16C:
                      ctxtH = cpool.tile([128, B * QW], BF)
                      ctxtL = cpool.tile([128, B * QW], BF)
                      nc.vector.tensor_copy(ctxtH[:], ctxt[:])
                      nc.vector.tensor_sub(ctxtL[:], ctxt[:], ctxtH[:])
                      lhs_list = [(ctxtH, 0), (ctxtH, 1), (ctxtL, 0)]
                  else:
                      lhs_list = [(ctxt, 0)]
                  NCH = D // 512  # 8 output column chunks
                  for n in range(4, NCH):
                      issue_wo(n)
                  for n in range(NCH):
                      wo_t = wo_tiles[n]
                      op_t = pwo.tile([128, 512], F32, tag="o", name=f"o{n}")
                      for ci, (lt, w) in enumerate(lhs_list):
                          for h in range(HQ):
                              nc.tensor.matmul(
                                  op_t[:], lt[:, h * 128:(h + 1) * 128],
                                  wo_t[:, w * WOW + h * 512:w * WOW + (h + 1) * 512],
                                  start=(ci == 0 and h == 0),
                                  stop=(ci == len(lhs_list) - 1 and h == HQ - 1))
                      ot = outp.tile([128, 512], F32, tag="ot", name=f"ot{n}")
                      nc.vector.tensor_copy(ot[:], op_t[:])
                      nc.sync.dma_start(out_d[:, n * 512:(n + 1) * 512], ot[:])

    nc.compile()
    return nc


def _prep_host(x, wq, wk, wv, wo, cos, sin, cache_k, cache_v, L):
    """Pack full inputs into per-core DMA-friendly slabs."""
    K_BF16C = USE_BF16C
    B, S, _ = x.shape
    T = B * S
    nfull, rem = divmod(L, 128)
    tpb = nfull + (1 if rem else 0)

    f = np.float32
    bf = mybir.dt.np(mybir.dt.bfloat16)

    def hilo(a):
        # interleave per-chunk [hi | lo] along axis 1 at chunk granularity is
        # done by the callers; here: full-width hi/lo halves
        hi = a.astype(bf)
        lo = (a - hi.astype(f)).astype(bf)
        return hi, lo

    x_flat = np.ascontiguousarray(np.asarray(x, f).reshape(T, D))
    xt = np.ascontiguousarray(
        x_flat.reshape(T, D // 128, 128).transpose(2, 1, 0).reshape(128, -1))

    cs_ = np.asarray(cos, f)[L:L + S]
    sn_ = np.asarray(sin, f)[L:L + S]
    crep = np.ascontiguousarray(
        np.tile(np.tile(np.repeat(cs_, 2, axis=1), (B, 1)), (1, HQ + 1)))
    shalf = np.ascontiguousarray(np.tile(np.tile(sn_, (B, 1)), (1, HQ + 1)))
    ident = np.eye(128, dtype=f)

    wq = np.asarray(wq, f)
    wk = np.asarray(wk, f)
    wv = np.asarray(wv, f)
    wo = np.asarray(wo, f)
    cache_k = np.asarray(cache_k, f)
    cache_v = np.asarray(cache_v, f)

    if K_BF16C:
        xth, xtl = hilo(xt)
        xt = np.ascontiguousarray(np.concatenate(
            [np.concatenate([xth[:, c * 1024:(c + 1) * 1024],
                             xtl[:, c * 1024:(c + 1) * 1024]], axis=1)
             for c in range(4)], axis=1))
    shared = dict(xt=xt, crep=crep, shalf=shalf, ident=ident)
    in_maps = []
    for c in range(NCORES):
        wq_c = wq[:, c * CW:(c + 1) * CW]
        wq_l = np.ascontiguousarray(
            wq_c.reshape(D // 128, 128, CW).transpose(1, 0, 2).reshape(128, -1))
        wkv_c = np.concatenate(
            [wk[:, c * HD:(c + 1) * HD], wv[:, c * HD:(c + 1) * HD]], axis=1)
        wkv_l = np.ascontiguousarray(
            wkv_c.reshape(D // 128, 128, KVW).transpose(1, 0, 2).reshape(128, -1))
        wo_c = wo[c * CW:(c + 1) * CW, :]
        # [p, n*2048 + h*512 + nn] = wo_c[h*128 + p, n*512 + nn]
        wo_l = np.ascontiguousarray(
            wo_c.reshape(HQ, 128, D // 512, 512).transpose(1, 2, 0, 3).reshape(128, -1))
        if K_BF16C:
            def chunked_hilo(a, nch):
                w = a.shape[1] // nch
                hi, lo = hilo(a)
                return np.ascontiguousarray(np.concatenate(
                    [np.concatenate([hi[:, i * w:(i + 1) * w],
                                     lo[:, i * w:(i + 1) * w]], axis=1)
                     for i in range(nch)], axis=1))
            wq_l = chunked_hilo(wq_l, 4)    # 4 weight chunks
            wkv_l = chunked_hilo(wkv_l, 4)
            wo_l = chunked_hilo(wo_l, 8)    # 8 column chunks
        m = dict(shared, wq=wq_l, wkv=wkv_l, wo=wo_l)
        if tpb:
            kpad = np.zeros((B, tpb * 128, 128), f)
            kpad[:, :L] = cache_k[:, :L, c, :]
            kl = np.ascontiguousarray(
                kpad.reshape(B, tpb, 128, 128).transpose(3, 0, 1, 2).reshape(128, -1))
            if K_BF16C:
                kh, klo = hilo(kl)
                nchk = B * tpb
                kl = np.ascontiguousarray(np.concatenate(
                    [np.concatenate([kh[:, i*128:(i+1)*128],
                                     klo[:, i*128:(i+1)*128]], axis=1)
                     for i in range(nchk)], axis=1))
            m["keyst"] = kl
            vpad = np.zeros((B, tpb * 128, HD + 1), f)
            vpad[:, :L, :HD] = cache_v[:, :L, c, :]
            vpad[:, :L, HD] = 1.0
            vl = np.ascontiguousarray(
                vpad.reshape(B, tpb, 128, HD + 1).transpose(2, 0, 1, 3).reshape(128, -1))
            if K_BF16C:
                # per (b,j) chunk: [hi(129) | lo(129)] adjacent
                vh, vlo = hilo(vl)
                nchv = B * tpb
                vl = np.ascontiguousarray(np.concatenate(
                    [np.concatenate([vh[:, i*(HD+1):(i+1)*(HD+1)],
                                     vlo[:, i*(HD+1):(i+1)*(HD+1)]], axis=1)
                     for i in range(nchv)], axis=1))
            m["vals"] = vl
        in_maps.append(m)
    return in_maps


def kernel(x, wq, wk, wv, wo, cos, sin, cache_k, cache_v, start_pos):
    global _LAST
    B, S, _ = x.shape
    L = int(start_pos)

    key = (L, B, S)
    if key not in _BUILD_CACHE:
        _BUILD_CACHE[key] = _build(L, B, S)
    nc = _BUILD_CACHE[key]

    in_maps = _prep_host(x, wq, wk, wv, wo, cos, sin, cache_k, cache_v, L)
    res = bass_utils.run_bass_kernel_spmd(
        nc, in_maps, core_ids=list(range(NCORES)),
        trace=TRACE or bool(os.environ.get("BASS_TRACE")))
    _LAST = res
    out = np.zeros((B * S, D), np.float32)
    for r in res.results:
        out += np.asarray(r["out"], np.float32)
    return out.reshape(B, S, D)



# revision 17
# speedup vs baseline: 1.8807x; 1.8807x over previous
"""GQA attention decode step (B=8, S=16, D=4096, H=32, KVH=8, HD=128) on 8
Trainium2 NeuronCores, tensor-parallel over heads.

Core c owns q-heads [4c, 4c+4), kv-head c, wo rows [512c, 512(c+1)).
Each core computes a full (128, 4096) partial output; the host sums the 8
partials (the all-reduce) and reshapes to (B, S, D).
"""

import os
import sys

import numpy as np

sys.path.insert(0, "/opt/trn_rl_repo")

import concourse.bacc as bacc  # noqa: E402
import concourse.mybir as mybir  # noqa: E402
import concourse.tile as tile  # noqa: E402
from concourse.tile import add_dep_helper  # noqa: E402
from concourse import bass_utils  # noqa: E402

D = 4096
H = 32
KVH = 8
HD = 128
NCORES = 8
HQ = H // NCORES          # 4 q heads per core
CW = HQ * HD              # 512 = per-core q width / wo row count
KVW = 2 * HD              # 256 = fused [k|v] projection width

TRACE = False
# "bf16c": hi/lo bf16 compensated matmuls (f32-equivalent bytes, 3 passes)
# "bf16":  pure bf16 weights/cache/activations (half the DMA bytes, 1 pass)
# "f32":   plain fp32
MODE = "bf16"
USE_BF16C = MODE == "bf16c"
_LAST = None              # last BassKernelResults (for test.py)
_BUILD_CACHE = {}

F32 = mybir.dt.float32


def _build(L, B, S, reps=1):
    """Build the per-core Bass program (identical across cores; SPMD).

    reps>1 replicates the whole body (for on-device benchmarking: one
    dispatch runs the kernel reps times back-to-back)."""
    T = B * S
    assert T == 128, "kernel assumes 128 tokens (B*S)"
    nfull, rem = divmod(L, 128)
    CS = [128] * nfull + ([rem] if rem else [])  # cache kv-chunk sizes
    ncache = len(CS)
    tpb = ncache                                  # cache tiles per batch
    NCHUNK = ncache + 1                           # + new-kv chunk (S wide)
    QW = HQ * S                                   # 64 score columns (4 heads x 16 tok)
    SCW = NCHUNK * QW                             # scoresT psum width per batch
    NKT = D // 128                                # 32 contraction tiles
    WCH = 8                                       # k-tiles per weight DMA chunk
    NWCH = NKT // WCH                             # 4 weight chunks
    scale = 1.0 / float(np.sqrt(HD))

    nc = bacc.Bacc("TRN2", target_bir_lowering=False, debug=False)
    BF = mybir.dt.bfloat16
    WMUL = 2 if USE_BF16C else 1
    WT = F32 if MODE == "f32" else BF

    xt_d = nc.dram_tensor("xt", (128, WMUL * D), WT, kind="ExternalInput")
    wq_d = nc.dram_tensor("wq", (128, WMUL * NKT * CW), WT, kind="ExternalInput")
    wkv_d = nc.dram_tensor("wkv", (128, WMUL * NKT * KVW), WT, kind="ExternalInput")
    wo_d = nc.dram_tensor("wo", (128, WMUL * HQ * D), WT, kind="ExternalInput")
    crep_d = nc.dram_tensor("crep", (128, (HQ + 1) * HD), F32, kind="ExternalInput")
    shalf_d = nc.dram_tensor("shalf", (128, (HQ + 1) * HD // 2), F32, kind="ExternalInput")
    ident_d = nc.dram_tensor("ident", (128, 128), F32, kind="ExternalInput")
    if ncache:
        keyst_d = nc.dram_tensor("keyst", (128, WMUL * B * tpb * 128), WT, kind="ExternalInput")
        vals_d = nc.dram_tensor("vals", (128, WMUL * B * tpb * (HD + 1)), WT, kind="ExternalInput")
    OUTT = BF if MODE == "bf16" else F32
    out_d = nc.dram_tensor("out", (T, D), OUTT, kind="ExternalOutput")

    with tile.TileContext(nc) as tc:
      for _rep in range(reps):
        with tc.tile_pool(name=f"const{_rep}", bufs=1) as cpool:
            crep = cpool.tile([128, (HQ + 1) * HD], F32)
            nc.sync.dma_start(crep[:], crep_d[:])
            shalf = cpool.tile([128, (HQ + 1) * HD // 2], F32)
            nc.sync.dma_start(shalf[:], shalf_d[:])
            ident = cpool.tile([128, 128], F32)
            nc.sync.dma_start(ident[:], ident_d[:])
            # long-lived intermediates
            qt = cpool.tile([128, CW], F32)            # (hd, h*128 + tok)
            kt = cpool.tile([128, 128], F32)           # (hd, tok)
            vreb = cpool.tile([S, B * (HD + 1)], F32)  # new-v, partition-rebased
            vrebH = cpool.tile([S, B * (HD + 1)], BF)
            vrebL = cpool.tile([S, B * (HD + 1)], BF)
            qtH = cpool.tile([128, CW], BF)
            qtL = cpool.tile([128, CW], BF)
            ktH = cpool.tile([128, 128], BF)
            ktL = cpool.tile([128, 128], BF)
            ctxt = cpool.tile([128, B * QW], F32)      # (hd, b*64 + h*16 + s)

            # persistent kv-cache tiles: allocated up-front (SBUF-disjoint from
            # the transient weight pools) so their DMAs can issue right after
            # the weight stream with no WAR waits
            kc_tiles, vc_tiles = [], []
            if ncache:
                KBW = WMUL * 128
                VBW = WMUL * (HD + 1)
                for b in range(B):
                    kc_tiles.append(cpool.tile([128, tpb * KBW], WT, name=f"kcT{b}"))
                    vc_tiles.append(cpool.tile([128, tpb * VBW], WT, name=f"vcT{b}"))
            kv_last_inst = None

            # ---------------- phase 1: qkv projections ----------------
            with (
                tc.tile_pool(name=f"xtp{_rep}", bufs=4) as xtp,
                tc.tile_pool(name=f"wqp{_rep}", bufs=4) as wqp,
                tc.tile_pool(name=f"wkvp{_rep}", bufs=4) as wkvp,
                tc.tile_pool(name="pqkv", bufs=1, space="PSUM") as pqkv,
            ):
                q_ps = pqkv.tile([128, CW], F32, tag="q")
                kv_ps = pqkv.tile([128, KVW], F32, tag="kv")
                # hi/lo combos: (xtH,wH), (xtH,wL), (xtL,wH); lo halves sit in
                # the second half of each chunk's columns
                combos = [(0, 0), (0, 1), (1, 0)] if USE_BF16C else [(0, 0)]
                XW, QWW, KWW = WCH * 128, WCH * CW, WCH * KVW
                for ch in range(NWCH):
                    xt_t = xtp.tile([128, WMUL * XW], WT, tag="xt")
                    nc.sync.dma_start(xt_t[:], xt_d[:, ch * WMUL * XW:(ch + 1) * WMUL * XW])
                    wq_t = wqp.tile([128, WMUL * QWW], WT, tag="wq")
                    nc.sync.dma_start(wq_t[:], wq_d[:, ch * WMUL * QWW:(ch + 1) * WMUL * QWW])
                    wkv_t = wkvp.tile([128, WMUL * KWW], WT, tag="wkv")
                    nc.sync.dma_start(wkv_t[:], wkv_d[:, ch * WMUL * KWW:(ch + 1) * WMUL * KWW])
                    for j in range(WCH):
                        k = ch * WCH + j
                        for ci, (a, w) in enumerate(combos):
                            first = (k == 0 and ci == 0)
                            last = (k == NKT - 1 and ci == len(combos) - 1)
                            xs = a * XW + j * 128
                            nc.tensor.matmul(
                                q_ps[:], xt_t[:, xs:xs + 128],
                                wq_t[:, w * QWW + j * CW:w * QWW + (j + 1) * CW],
                                start=first, stop=last)
                            nc.tensor.matmul(
                                kv_ps[:], xt_t[:, xs:xs + 128],
                                wkv_t[:, w * KWW + j * KVW:w * KWW + (j + 1) * KVW],
                                start=first, stop=last)

                # kv-cache stream: issued here (after the weight DMAs in SP
                # program order) so the DMA engines stay busy through the
                # rotary/transpose phases
                if ncache:
                    for b in range(B):
                        nc.sync.dma_start(
                            kc_tiles[b][:],
                            keyst_d[:, b * tpb * KBW:(b + 1) * tpb * KBW])
                        kv_last_inst = nc.sync.dma_start(
                            vc_tiles[b][:],
                            vals_d[:, b * tpb * VBW:(b + 1) * tpb * VBW])

                # ---------------- phase 2: rotary (DVE) ----------------
                # rot = t*crep ; rot_even -= t_odd*shalf ; rot_odd += t_even*shalf
                with tc.tile_pool(name=f"rotp{_rep}", bufs=1) as rotp:
                    W = (HQ + 1) * HD  # 640 = 4 q heads + 1 k head
                    rot = rotp.tile([128, W], F32, tag="rot")
                    tmpa = rotp.tile([128, W // 2], F32, tag="tmpa")
                    tmpb = rotp.tile([128, W // 2], F32, tag="tmpb")
                    nc.vector.tensor_mul(rot[:, 0:CW], q_ps[:], crep[:, 0:CW])
                    nc.vector.tensor_mul(rot[:, CW:W], kv_ps[:, 0:HD], crep[:, CW:W])
                    q_v = q_ps[:].rearrange("p (a two) -> p a two", two=2)
                    k_v = kv_ps[:, 0:HD].rearrange("p (a two) -> p a two", two=2)
                    nc.vector.tensor_mul(tmpa[:, 0:CW // 2], q_v[:, :, 1], shalf[:, 0:CW // 2])
                    nc.vector.tensor_mul(tmpa[:, CW // 2:], k_v[:, :, 1], shalf[:, CW // 2:])
                    nc.vector.tensor_mul(tmpb[:, 0:CW // 2], q_v[:, :, 0], shalf[:, 0:CW // 2])
                    nc.vector.tensor_mul(tmpb[:, CW // 2:], k_v[:, :, 0], shalf[:, CW // 2:])
                    rot_v = rot[:].rearrange("p (a two) -> p a two", two=2)
                    nc.vector.tensor_sub(rot_v[:, :, 0], rot_v[:, :, 0], tmpa[:])
                    nc.vector.tensor_add(rot_v[:, :, 1], rot_v[:, :, 1], tmpb[:])

                    # -------- phase 3: transposes + new-v staging --------
                    vaug = rotp.tile([128, HD + 1], F32, tag="vaug")
                    # qt is stored batch-major: column b*64 + h*16 + s, so the
                    # scores rhs is a contiguous (128, 64) slice per batch
                    qt_v = qt[:].rearrange("p (b h s) -> p b h s", b=B, h=HQ)
                    vt = rotp.tile([128, 128], F32, tag="vt")
                    with tc.tile_pool(name=f"ptr{_rep}", bufs=2, space="PSUM") as ptr:
                        for h in range(HQ):
                            tp = ptr.tile([128, 128], F32, tag="tr", name=f"tr{h}")
                            nc.tensor.transpose(tp[:], rot[:, h * 128:(h + 1) * 128], ident[:])
                            nc.vector.tensor_copy(
                                qt_v[:, :, h, :],
                                tp[:].rearrange("p (b s) -> p b s", b=B))
                        tp = ptr.tile([128, 128], F32, tag="tr", name="trk")
                        nc.tensor.transpose(tp[:], rot[:, CW:W], ident[:])
                        nc.vector.tensor_copy(kt[:], tp[:])
                        # new-v rebased to partition 0 via double transpose (no
                        # DMA: the sbuf->sbuf route queues behind ~20MB of
                        # loads and stalls PV(0) -> in-order PE for ~14us)
                        nc.vector.tensor_copy(vaug[:, 0:HD], kv_ps[:, HD:KVW])
                        nc.vector.memset(vaug[:, HD:HD + 1], 1.0)
                        tpv = ptr.tile([128, 128], F32, tag="tr", name="trv")
                        nc.tensor.transpose(tpv[:], vaug[:, 0:HD], ident[:])
                        nc.vector.tensor_copy(vt[:], tpv[:])  # (hd, tok)
                        for b in range(B):
                            tq = ptr.tile([S, 128], F32, tag="trb", name=f"trb{b}")
                            nc.tensor.transpose(tq[:], vt[:, b * S:(b + 1) * S], ident[:])
                            nc.vector.tensor_copy(
                                vreb[:, b * (HD + 1):b * (HD + 1) + HD], tq[:])
                            nc.vector.memset(
                                vreb[:, b * (HD + 1) + HD:(b + 1) * (HD + 1)], 1.0)
                        nc.vector.tensor_copy(vrebH[:], vreb[:])
                        nc.vector.tensor_copy(qtH[:], qt[:])
                        nc.vector.tensor_copy(ktH[:], kt[:])
                        if USE_BF16C:
                            nc.vector.tensor_sub(vrebL[:], vreb[:], vrebH[:])
                            nc.vector.tensor_sub(qtL[:], qt[:], qtH[:])
                            nc.vector.tensor_sub(ktL[:], kt[:], ktH[:])

            # ---------------- phase 4: attention per batch ----------------
            # wop opens alongside the attention pools so its SBUF space is
            # disjoint from kc/vc (else wo DMAs inherit WAR deps on them)
            with tc.tile_pool(name=f"wop{_rep}", bufs=8) as wop:
              with (
                  tc.tile_pool(name=f"expp{_rep}", bufs=2) as expp,
                  tc.tile_pool(name=f"psc{_rep}", bufs=2, space="PSUM") as psc,
                  tc.tile_pool(name=f"pctx{_rep}", bufs=2, space="PSUM") as pctx,
                  tc.tile_pool(name=f"pctr{_rep}", bufs=2, space="PSUM") as pctr,
              ):
                  wo_tiles = {}
                  last_kv = {"inst": kv_last_inst}
                  WOW = HQ * 512
                  NCH_WO = D // 512

                  def issue_wo(n):
                      wo_t = wop.tile([128, WMUL * WOW], WT, tag="wo", name=f"wo{n}")
                      di = nc.sync.dma_start(
                          wo_t[:], wo_d[:, n * WMUL * WOW:(n + 1) * WMUL * WOW])
                      if last_kv["inst"] is not None:
                          # strict back-pressure: wo bandwidth must not delay
                          # the kv stream (lanes bandwidth-share otherwise)
                          add_dep_helper(di.ins, last_kv["inst"].ins,
                                         reason="wo after last kv")
                      wo_tiles[n] = wo_t

                  st = {}

                  def do_pv(i):
                      # PV + normalize for batch i (exp already done), bf16
                      # hi/lo compensated: eH@vH + eH@vL + eL@vH
                      exHi, exLi, vci = st[i]["exH"], st[i]["exL"], st[i]["vc"]
                      VB = WMUL * (HD + 1)
                      ctx = pctx.tile([QW, HD + 1], F32, tag="ctx", name=f"ctx{i}")
                      pv_combos = [(exHi, 0), (exHi, 1), (exLi, 0)] if USE_BF16C \
                          else [(exHi, 0)]
                      first = True
                      for j, cs in enumerate(CS):
                          for el, w in pv_combos:
                              nc.tensor.matmul(
                                  ctx[:], el[0:cs, j * QW:(j + 1) * QW],
                                  vci[0:cs, j * VB + w * (HD + 1):
                                      j * VB + (w + 1) * (HD + 1)],
                                  start=first, stop=False)
                              first = False
                      vreb_combos = [(exHi, vrebH), (exHi, vrebL), (exLi, vrebH)] \
                          if USE_BF16C else [(exHi, vrebH if MODE == "bf16" else vreb)]
                      for ci, (el, vr) in enumerate(vreb_combos):
                          nc.tensor.matmul(
                              ctx[:], el[0:S, ncache * QW:NCHUNK * QW],
                              vr[:, i * (HD + 1):(i + 1) * (HD + 1)],
                              start=(ncache == 0 and ci == 0),
                              stop=(ci == len(vreb_combos) - 1))
                      rc = expp.tile([QW, 1], F32, tag="rc", name=f"rc{i}")
                      nc.vector.reciprocal(rc[:], ctx[:, HD:HD + 1])
                      cn = expp.tile([QW, HD], F32, tag="cn", name=f"cn{i}")
                      nc.vector.tensor_scalar_mul(cn[:], ctx[:, 0:HD], rc[:])
                      st[i]["cn"] = cn

                  def do_tr(i):
                      # transpose + scatter into ctxt for batch i
                      ct = pctr.tile([128, QW], F32, tag="ct", name=f"ct{i}")
                      nc.tensor.transpose(ct[:], st[i]["cn"][:], ident[0:QW, 0:QW])
                      nc.vector.tensor_copy(
                          ctxt[:].rearrange("p (h b s) -> p h b s", h=HQ, b=B)[:, :, i, :],
                          ct[:].rearrange("p (h s) -> p h s", h=HQ))

                  for b in range(B):
                      if ncache:
                          kc_t = kc_tiles[b]
                          vc_t = vc_tiles[b]
                      else:
                          kc_t = vc_t = None
                      if b == B - 1:
                          # wo stream starts after ALL kv: just-in-time arrival
                          for n in range(4):
                              issue_wo(n)
                      qbH = qtH[:, b * QW:(b + 1) * QW]
                      qbL = qtL[:, b * QW:(b + 1) * QW]

                      sc = psc.tile([128, SCW], F32, tag="sc", name=f"sc{b}")
                      # pre-fill columns of partial chunks so garbage partitions
                      # exp() to 0; the matmuls below overwrite the valid rows
                      nc.vector.memset(sc[:, ncache * QW:NCHUNK * QW], -1e30)
                      for j, cs in enumerate(CS):
                          if cs < 128:
                              nc.vector.memset(sc[:, j * QW:(j + 1) * QW], -1e30)
                      KB = WMUL * 128
                      for j, cs in enumerate(CS):
                          combos = [(0, qbH), (0, qbL), (1, qbH)] if USE_BF16C \
                              else [(0, qbH)]
                          for ci, (w, qb) in enumerate(combos):
                              nc.tensor.matmul(
                                  sc[0:cs, j * QW:(j + 1) * QW],
                                  kc_t[:, j * KB + w * 128:j * KB + w * 128 + cs],
                                  qb,
                                  start=(ci == 0), stop=(ci == len(combos) - 1))
                      # new-kv chunk
                      ncomb = [(ktH, qbH), (ktH, qbL), (ktL, qbH)] if USE_BF16C \
                          else [(ktH, qbH)]
                      for ci, (ktx, qb) in enumerate(ncomb):
                          nc.tensor.matmul(
                              sc[0:S, ncache * QW:NCHUNK * QW],
                              ktx[:, b * S:(b + 1) * S], qb,
                              start=(ci == 0), stop=(ci == len(ncomb) - 1))

                      # software pipeline: PE stays busy with prior batches'
                      # PV/transpose while ACT runs this batch's exp
                      if b >= 1:
                          do_pv(b - 1)
                      if b >= 2:
                          do_tr(b - 2)

                      ex = expp.tile([128, SCW], F32, tag="ex", name=f"ex{b}")
                      nc.scalar.activation(ex[:], sc[:], mybir.ActivationFunctionType.Exp,
                                           scale=scale)
                      exH = expp.tile([128, SCW], BF, tag="exH", name=f"exH{b}")
                      nc.vector.tensor_copy(exH[:], ex[:])
                      if USE_BF16C:
                          exL = expp.tile([128, SCW], BF, tag="exL", name=f"exL{b}")
                          nc.vector.tensor_sub(exL[:], ex[:], exH[:])
                      else:
                          exL = None
                      st[b] = dict(exH=exH, exL=exL, vc=vc_t)

                  do_pv(B - 1)
                  do_tr(B - 2)
                  do_tr(B - 1)

              # ---------------- phase 5: wo projection ----------------
              # wo is chunked by OUTPUT columns: each 1MB chunk holds all 4
              # h-blocks for one 512-col output slice, so its psum completes
              # and streams out immediately
              with (
                  tc.tile_pool(name=f"outp{_rep}", bufs=4) as outp,
                  tc.tile_pool(name=f"pwo{_rep}", bufs=3, space="PSUM") as pwo,
              ):
                  if USE_BF16C:
                      ctxtH = cpool.tile([128, B * QW], BF)
                      ctxtL = cpool.tile([128, B * QW], BF)
                      nc.vector.tensor_copy(ctxtH[:], ctxt[:])
                      nc.vector.tensor_sub(ctxtL[:], ctxt[:], ctxtH[:])
                      lhs_list = [(ctxtH, 0), (ctxtH, 1), (ctxtL, 0)]
                  elif MODE == "bf16":
                      ctxtH = cpool.tile([128, B * QW], BF)
                      nc.vector.tensor_copy(ctxtH[:], ctxt[:])
                      lhs_list = [(ctxtH, 0)]
                  else:
                      lhs_list = [(ctxt, 0)]
                  NCH = D // 512  # 8 output column chunks
                  for n in range(4, NCH):
                      issue_wo(n)
                  for n in range(NCH):
                      wo_t = wo_tiles[n]
                      op_t = pwo.tile([128, 512], F32, tag="o", name=f"o{n}")
                      for ci, (lt, w) in enumerate(lhs_list):
                          for h in range(HQ):
                              nc.tensor.matmul(
                                  op_t[:], lt[:, h * 128:(h + 1) * 128],
                                  wo_t[:, w * WOW + h * 512:w * WOW + (h + 1) * 512],
                                  start=(ci == 0 and h == 0),
                                  stop=(ci == len(lhs_list) - 1 and h == HQ - 1))
                      ot = outp.tile([128, 512], OUTT, tag="ot", name=f"ot{n}")
                      nc.vector.tensor_copy(ot[:], op_t[:])
                      nc.sync.dma_start(out_d[:, n * 512:(n + 1) * 512], ot[:])

    nc.compile()
    return nc


def _prep_host(x, wq, wk, wv, wo, cos, sin, cache_k, cache_v, L):
    """Pack full inputs into per-core DMA-friendly slabs."""
    K_BF16C = USE_BF16C
    B, S, _ = x.shape
    T = B * S
    nfull, rem = divmod(L, 128)
    tpb = nfull + (1 if rem else 0)

    f = np.float32
    bf = mybir.dt.np(mybir.dt.bfloat16)

    def hilo(a):
        # interleave per-chunk [hi | lo] along axis 1 at chunk granularity is
        # done by the callers; here: full-width hi/lo halves
        hi = a.astype(bf)
        lo = (a - hi.astype(f)).astype(bf)
        return hi, lo

    x_flat = np.ascontiguousarray(np.asarray(x, f).reshape(T, D))
    xt = np.ascontiguousarray(
        x_flat.reshape(T, D // 128, 128).transpose(2, 1, 0).reshape(128, -1))

    cs_ = np.asarray(cos, f)[L:L + S]
    sn_ = np.asarray(sin, f)[L:L + S]
    crep = np.ascontiguousarray(
        np.tile(np.tile(np.repeat(cs_, 2, axis=1), (B, 1)), (1, HQ + 1)))
    shalf = np.ascontiguousarray(np.tile(np.tile(sn_, (B, 1)), (1, HQ + 1)))
    ident = np.eye(128, dtype=f)

    wq = np.asarray(wq, f)
    wk = np.asarray(wk, f)
    wv = np.asarray(wv, f)
    wo = np.asarray(wo, f)
    cache_k = np.asarray(cache_k, f)
    cache_v = np.asarray(cache_v, f)

    if K_BF16C:
        xth, xtl = hilo(xt)
        xt = np.ascontiguousarray(np.concatenate(
            [np.concatenate([xth[:, c * 1024:(c + 1) * 1024],
                             xtl[:, c * 1024:(c + 1) * 1024]], axis=1)
             for c in range(4)], axis=1))
    elif MODE == "bf16":
        xt = np.ascontiguousarray(xt.astype(bf))
    shared = dict(xt=xt, crep=crep, shalf=shalf, ident=ident)
    in_maps = []
    for c in range(NCORES):
        wq_c = wq[:, c * CW:(c + 1) * CW]
        wq_l = np.ascontiguousarray(
            wq_c.reshape(D // 128, 128, CW).transpose(1, 0, 2).reshape(128, -1))
        wkv_c = np.concatenate(
            [wk[:, c * HD:(c + 1) * HD], wv[:, c * HD:(c + 1) * HD]], axis=1)
        wkv_l = np.ascontiguousarray(
            wkv_c.reshape(D // 128, 128, KVW).transpose(1, 0, 2).reshape(128, -1))
        wo_c = wo[c * CW:(c + 1) * CW, :]
        # [p, n*2048 + h*512 + nn] = wo_c[h*128 + p, n*512 + nn]
        wo_l = np.ascontiguousarray(
            wo_c.reshape(HQ, 128, D // 512, 512).transpose(1, 2, 0, 3).reshape(128, -1))
        if K_BF16C:
            def chunked_hilo(a, nch):
                w = a.shape[1] // nch
                hi, lo = hilo(a)
                return np.ascontiguousarray(np.concatenate(
                    [np.concatenate([hi[:, i * w:(i + 1) * w],
                                     lo[:, i * w:(i + 1) * w]], axis=1)
                     for i in range(nch)], axis=1))
            wq_l = chunked_hilo(wq_l, 4)    # 4 weight chunks
            wkv_l = chunked_hilo(wkv_l, 4)
            wo_l = chunked_hilo(wo_l, 8)    # 8 column chunks
        elif MODE == "bf16":
            wq_l = np.ascontiguousarray(wq_l.astype(bf))
            wkv_l = np.ascontiguousarray(wkv_l.astype(bf))
            wo_l = np.ascontiguousarray(wo_l.astype(bf))
        m = dict(shared, wq=wq_l, wkv=wkv_l, wo=wo_l)
        if tpb:
            kpad = np.zeros((B, tpb * 128, 128), f)
            kpad[:, :L] = cache_k[:, :L, c, :]
            kl = np.ascontiguousarray(
                kpad.reshape(B, tpb, 128, 128).transpose(3, 0, 1, 2).reshape(128, -1))
            if K_BF16C:
                kh, klo = hilo(kl)
                nchk = B * tpb
                kl = np.ascontiguousarray(np.concatenate(
                    [np.concatenate([kh[:, i*128:(i+1)*128],
                                     klo[:, i*128:(i+1)*128]], axis=1)
                     for i in range(nchk)], axis=1))
            elif MODE == "bf16":
                kl = np.ascontiguousarray(kl.astype(bf))
            m["keyst"] = kl
            vpad = np.zeros((B, tpb * 128, HD + 1), f)
            vpad[:, :L, :HD] = cache_v[:, :L, c, :]
            vpad[:, :L, HD] = 1.0
            vl = np.ascontiguousarray(
                vpad.reshape(B, tpb, 128, HD + 1).transpose(2, 0, 1, 3).reshape(128, -1))
            if K_BF16C:
                # per (b,j) chunk: [hi(129) | lo(129)] adjacent
                vh, vlo = hilo(vl)
                nchv = B * tpb
                vl = np.ascontiguousarray(np.concatenate(
                    [np.concatenate([vh[:, i*(HD+1):(i+1)*(HD+1)],
                                     vlo[:, i*(HD+1):(i+1)*(HD+1)]], axis=1)
                     for i in range(nchv)], axis=1))
            elif MODE == "bf16":
                vl = np.ascontiguousarray(vl.astype(bf))
            m["vals"] = vl
        in_maps.append(m)
    return in_maps


def kernel(x, wq, wk, wv, wo, cos, sin, cache_k, cache_v, start_pos):
    global _LAST
    B, S, _ = x.shape
    L = int(start_pos)

    key = (L, B, S)
    if key not in _BUILD_CACHE:
        _BUILD_CACHE[key] = _build(L, B, S)
    nc = _BUILD_CACHE[key]

    in_maps = _prep_host(x, wq, wk, wv, wo, cos, sin, cache_k, cache_v, L)
    res = bass_utils.run_bass_kernel_spmd(
        nc, in_maps, core_ids=list(range(NCORES)),
        trace=TRACE or bool(os.environ.get("BASS_TRACE")))
    _LAST = res
    out = np.zeros((B * S, D), np.float32)
    for r in res.results:
        out += np.asarray(r["out"], np.float32)
    return out.reshape(B, S, D)

